# revision 12
# baseline (speedup 1.0000x reference)
"""Multi-head attention Trainium2 Bass kernel.

Problem: x[8,1024,768], qkv_w[2304,768], qkv_b[2304], proj_w[768,768],
proj_b[768] -> out[8,1024,768]  (12 heads, head_dim 64, softmax scale 1/8).

Sharding: data-parallel over the batch dim — one batch element per
NeuronCore, 8 cores, no collectives.

Per-core pipeline (all matmul inputs bf16, fp32 PSUM accumulation):
  1. Load x / weights fp32, cast bf16 (ACT), DMA-transpose to
     xT[c,n], wT[c,d'], pwT[c,c_out].
  2. QKV: Q,K produced transposed (qT/kT [d,n]) with per-partition bias;
     V produced natural [n,d] with a ones column appended per head.
  3. Per head: scores sT[j,i] = kT.T @ qT (head pairs at partition bases
     0/64 run row-packed concurrently on the PE); exp on ACT with the
     1/8 softmax scale folded in (scores absmax ~2.7, no max-sub needed);
     pv: outT[d+1, i] += [v|1].T @ exp_sT accumulated over j-tiles — the
     ones column yields the softmax denominator row for free.
  4. Batched reciprocal of the 12 denominator rows, broadcast via a DRAM
     bounce, one normalize multiply per head pair.
  5. proj: out[n, c_out] = attnT.T @ pwT (natural layout, no final
     transpose), bias add on DVE, DMA out.
"""

import sys

if "/opt/trn_rl_repo" not in sys.path:
    sys.path.insert(0, "/opt/trn_rl_repo")

from contextlib import ExitStack

import numpy as np

import concourse.bass as bass
import concourse.tile as tile
from concourse import mybir
from concourse.bass_utils import run_bass_kernel_spmd

F32 = mybir.dt.float32
BF16 = mybir.dt.bfloat16
AF = mybir.ActivationFunctionType


def _split_dma_waits(nc: bass.Bass):
    """TRN2 instruction encodings hold at most 1 sync-wait (EventSemaphore: 2),
    but Tile can attach several (producer + xbar-mode serialization guards).
    Hoist all but one wait onto single-wait NoOps inserted just before on the
    same engine — same-sequencer FIFO order makes this equivalent.
    """
    for f in nc.m.functions:
        for blk in f.blocks:
            insts = blk.instructions
            i = 0
            while i < len(insts):
                inst = insts[i]
                limit = 2 if isinstance(inst, mybir.InstEventSemaphore) else 1
                if (inst.sync_info is not None
                        and len(inst.sync_info.on_wait) > limit):
                    waits = list(inst.sync_info.on_wait)
                    inst.sync_info = mybir.SyncInfo(
                        on_wait=waits[-limit:],
                        on_update=list(inst.sync_info.on_update))
                    for w in waits[:-limit]:
                        nop = mybir.InstNoOp(
                            name=nc.get_next_instruction_name(),
                            ins=[], outs=[])
                        nop.engine = inst.engine
                        nop.sync_info = mybir.SyncInfo(
                            on_wait=[w], on_update=[])
                        insts.insert(i, nop)
                        i += 1
                i += 1

B, N, C = 8, 1024, 768
H, HD = 12, 64
D3 = 3 * C  # 2304
SCALE = HD ** -0.5
NT = N // 128   # 8  token tiles
CT = C // 128   # 6  channel tiles
QT = D3 // 128  # 18 qkv row tiles


def build_kernel(nc: bass.Bass):
    x = nc.dram_tensor("x", [N, C], F32, kind="ExternalInput").ap()
    qkv_w = nc.dram_tensor("qkv_w", [D3, C], F32, kind="ExternalInput").ap()
    qkv_b = nc.dram_tensor("qkv_b", [D3], F32, kind="ExternalInput").ap()
    proj_w = nc.dram_tensor("proj_w", [C, C], F32, kind="ExternalInput").ap()
    proj_b = nc.dram_tensor("proj_b", [C], F32, kind="ExternalInput").ap()
    out = nc.dram_tensor("out", [N, C], F32, kind="ExternalOutput").ap()

    def bcast_ap(src: bass.AP, parts: int) -> bass.AP:
        # partition-broadcast a 1-D DRAM row: ap [[0, parts], [1, n]]
        return bass.AP(tensor=src.tensor, offset=src.offset,
                       ap=[[0, parts], *src.ap])

    with tile.TileContext(nc) as tc, ExitStack() as ctx:
        consts = ctx.enter_context(tc.tile_pool(name="consts", bufs=1))
        stage = ctx.enter_context(tc.tile_pool(name="stage", bufs=4))
        expp = ctx.enter_context(tc.tile_pool(name="expp", bufs=4))
        outp = ctx.enter_context(tc.tile_pool(name="outp", bufs=3))
        ps_s = ctx.enter_context(tc.tile_pool(name="ps_s", bufs=4, space="PSUM"))
        ps_o = ctx.enter_context(tc.tile_pool(name="ps_o", bufs=2, space="PSUM"))
        dram = ctx.enter_context(tc.tile_pool(name="dram", bufs=1, space="DRAM"))

        # ---- persistent bf16 operands --------------------------------
        xT = consts.tile([128, CT, N], BF16)        # x.T   [c, n]
        wT = consts.tile([128, CT, D3], BF16)       # qkv_w.T [c, d']
        pwT = consts.tile([128, CT, C], BF16)       # proj_w.T [c, c_out]
        qTt = consts.tile([128, CT, N], BF16)       # q.T  [d, n] (+bias)
        kTt = consts.tile([128, CT, N], BF16)       # k.T  [d, n] (+bias)
        v_sb = consts.tile([128, NT, H, HD + 1], BF16)  # v natural + ones col
        attnU = consts.tile([128, CT, N], F32)      # unnormalized attn.T
        attnT = consts.tile([128, CT, N], BF16)     # normalized attn.T
        qkb = consts.tile([128, 2 * CT], F32)       # q,k bias per-partition
        vb_bc = consts.tile([128, C], F32)          # v bias bcast
        pjb_bc = consts.tile([128, C], F32)         # proj bias bcast
        recip_all = consts.tile([H, N], F32)        # 1/denominator per head
        dscratch = dram.tile([H, N], F32)           # DRAM bounce for bcast

        # ---- biases ---------------------------------------------------
        # q,k bias laid out [p, t]: d' = t*128 + p  (t in 0..11)
        nc.sync.dma_start(out=qkb, in_=qkv_b[0:2 * C].rearrange(
            "(t p) -> p t", p=128))
        nc.sync.dma_start(out=vb_bc, in_=bcast_ap(qkv_b[2 * C:D3], 128))
        nc.sync.dma_start(out=pjb_bc, in_=bcast_ap(proj_b, 128))
        nc.vector.memset(v_sb[:, :, :, HD:HD + 1], 1.0)

        # ---- load + cast + transpose x and weights -------------------
        for t in range(NT):
            xs = stage.tile([128, C], F32, tag="xs")
            nc.sync.dma_start(out=xs, in_=x[t * 128:(t + 1) * 128, :])
            xb = stage.tile([128, C], BF16, tag="xb")
            nc.scalar.activation(out=xb, in_=xs, func=AF.Copy)
            nc.sync.dma_start_transpose(
                out=xT[:, :, t * 128:(t + 1) * 128], in_=xb)
        for t in range(QT):
            ws = stage.tile([128, C], F32, tag="xs")
            nc.sync.dma_start(out=ws, in_=qkv_w[t * 128:(t + 1) * 128, :])
            wb = stage.tile([128, C], BF16, tag="xb")
            nc.scalar.activation(out=wb, in_=ws, func=AF.Copy)
            nc.sync.dma_start_transpose(
                out=wT[:, :, t * 128:(t + 1) * 128], in_=wb)
        for t in range(CT):
            ws = stage.tile([128, C], F32, tag="xs")
            nc.sync.dma_start(out=ws, in_=proj_w[t * 128:(t + 1) * 128, :])
            wb = stage.tile([128, C], BF16, tag="xb")
            nc.scalar.activation(out=wb, in_=ws, func=AF.Copy)
            nc.sync.dma_start_transpose(
                out=pwT[:, :, t * 128:(t + 1) * 128], in_=wb)

        # ---- QKV projection ------------------------------------------
        # Q and K transposed: qkvT[d', n] = wT.T @ xT, d' tiles 0..11
        for t in range(2 * CT):
            dst = qTt if t < CT else kTt
            tt = t % CT
            for ic in range(2):
                ps = ps_s.tile([128, 512], F32, tag="ps")
                for ct in range(CT):
                    nc.tensor.matmul(
                        ps,
                        lhsT=wT[:, ct, t * 128:(t + 1) * 128],
                        rhs=xT[:, ct, ic * 512:(ic + 1) * 512],
                        start=(ct == 0), stop=(ct == CT - 1))
                nc.vector.tensor_scalar_add(
                    out=dst[:, tt, ic * 512:(ic + 1) * 512],
                    in0=ps, scalar1=qkb[:, t:t + 1])
        # V natural: v[n, dv] = xT.T @ wT[:, :, 1536:2304]
        for t in range(NT):
            for lo, hi in ((0, 512), (512, 768)):
                psv = ps_s.tile([128, hi - lo], F32, tag="ps", name="psv")
                for ct in range(CT):
                    nc.tensor.matmul(
                        psv,
                        lhsT=xT[:, ct, t * 128:(t + 1) * 128],
                        rhs=wT[:, ct, 2 * C + lo:2 * C + hi],
                        start=(ct == 0), stop=(ct == CT - 1))
                nh = (hi - lo) // HD
                nc.vector.tensor_add(
                    out=v_sb[:, t, lo // HD:hi // HD, 0:HD],
                    in0=psv.rearrange("p (h d) -> p h d", h=nh),
                    in1=vb_bc[:, lo:hi].rearrange("p (h d) -> p h d", h=nh))

        # ---- attention (head pairs share a 128-partition tile) -------
        for h in range(H):
            t, base = h // 2, (h % 2) * 64
            o_ps = ps_o.tile([HD + 1, N], F32, tag="ops")
            for jt in range(NT):
                s_ps = [ps_s.tile([128, 512], F32, tag="ps", name=f"s_ps{i}")
                        for i in range(2)]
                e = expp.tile([128, N], BF16, tag="e")
                for ic in range(2):
                    nc.tensor.matmul(
                        s_ps[ic],
                        lhsT=kTt[base:base + 64, t, jt * 128:(jt + 1) * 128],
                        rhs=qTt[base:base + 64, t, ic * 512:(ic + 1) * 512],
                        start=True, stop=True)
                    nc.scalar.activation(
                        out=e[:, ic * 512:(ic + 1) * 512], in_=s_ps[ic],
                        func=AF.Exp, scale=SCALE)
                for ic in range(2):
                    nc.tensor.matmul(
                        o_ps[:, ic * 512:(ic + 1) * 512],
                        lhsT=v_sb[:, jt, h, :],
                        rhs=e[:, ic * 512:(ic + 1) * 512],
                        start=(jt == 0), stop=(jt == NT - 1))
            # unnormalized head output -> attnU rows [base, base+64)
            nc.vector.tensor_copy(
                out=attnU[base:base + 64, t, :], in_=o_ps[0:HD, :])
            # denominator row -> partition-64 staging -> recip_all[h, :]
            den = stage.tile([65, N], F32, tag="den")
            nc.scalar.activation(
                out=den[HD:HD + 1, :], in_=o_ps[HD:HD + 1, :], func=AF.Copy)
            nc.sync.dma_start(out=recip_all[h:h + 1, :],
                              in_=den[HD:HD + 1, :])

        # ---- normalize -----------------------------------------------
        nc.vector.reciprocal(out=recip_all, in_=recip_all)
        nc.sync.dma_start(out=dscratch, in_=recip_all)
        for t in range(CT):
            rbc = stage.tile([128, N], F32, tag="rbc")
            nc.sync.dma_start(out=rbc[0:64, :],
                              in_=bcast_ap(dscratch[2 * t, :], 64))
            nc.sync.dma_start(out=rbc[64:128, :],
                              in_=bcast_ap(dscratch[2 * t + 1, :], 64))
            nc.vector.tensor_mul(
                out=attnT[:, t, :], in0=attnU[:, t, :], in1=rbc)

        # ---- output projection ---------------------------------------
        for t in range(NT):
            osb = outp.tile([128, C], F32, tag="osb")
            for lo, hi in ((0, 512), (512, 768)):
                pso = ps_s.tile([128, hi - lo], F32, tag="ps", name="pso")
                for ct in range(CT):
                    nc.tensor.matmul(
                        pso,
                        lhsT=attnT[:, ct, t * 128:(t + 1) * 128],
                        rhs=pwT[:, ct, lo:hi],
                        start=(ct == 0), stop=(ct == CT - 1))
                nc.vector.tensor_add(
                    out=osb[:, lo:hi], in0=pso, in1=pjb_bc[:, lo:hi])
            nc.sync.dma_start(out=out[t * 128:(t + 1) * 128, :], in_=osb)

    _split_dma_waits(nc)
    return nc


_NC_CACHE = None


def _get_nc():
    global _NC_CACHE
    if _NC_CACHE is None:
        _NC_CACHE = build_kernel(
            bass.Bass("TRN2", target_bir_lowering=False, debug=False))
    return _NC_CACHE


def kernel(**inputs: np.ndarray) -> np.ndarray:
    nc = _get_nc()
    x = np.ascontiguousarray(inputs["x"], dtype=np.float32)
    shared = {
        "qkv_w": np.ascontiguousarray(inputs["qkv_w"], dtype=np.float32),
        "qkv_b": np.ascontiguousarray(inputs["qkv_b"], dtype=np.float32),
        "proj_w": np.ascontiguousarray(inputs["proj_w"], dtype=np.float32),
        "proj_b": np.ascontiguousarray(inputs["proj_b"], dtype=np.float32),
    }
    in_maps = [{"x": x[b], **shared} for b in range(B)]
    res = run_bass_kernel_spmd(nc, in_maps, core_ids=list(range(B)))
    return np.stack([r["out"] for r in res.results]).astype(np.float32)


if __name__ == "__main__":
    from reference import setup_inputs, reference

    inputs = {k: np.asarray(v) for k, v in setup_inputs().items()}
    got = kernel(**inputs)
    exp = np.asarray(reference(**inputs))
    err = np.abs(got - exp)
    print("abs err max:", err.max(), "ref absmax:", np.abs(exp).max())
    print("rel(absmax):", err.max() / np.abs(exp).max())


# revision 20
# speedup vs baseline: 1.0498x; 1.0498x over previous
"""Multi-head attention Trainium2 Bass kernel.

Problem: x[8,1024,768], qkv_w[2304,768], qkv_b[2304], proj_w[768,768],
proj_b[768] -> out[8,1024,768]  (12 heads, head_dim 64, softmax scale 1/8).

Sharding: data-parallel over the batch dim — one batch element per
NeuronCore, 8 cores, no collectives.

Per-core pipeline (all matmul inputs bf16, fp32 PSUM accumulation):
  1. Load x / weights fp32, cast bf16 (ACT), DMA-transpose to
     xT[c,n], wT[c,d'], pwT[c,c_out].
  2. QKV: Q,K produced transposed (qT/kT [d,n]) with per-partition bias;
     V produced natural [n,d] with a ones column appended per head.
  3. Per head: scores sT[j,i] = kT.T @ qT (head pairs at partition bases
     0/64 run row-packed concurrently on the PE); exp on ACT with the
     1/8 softmax scale folded in (scores absmax ~2.7, no max-sub needed);
     pv: outT[d+1, i] += [v|1].T @ exp_sT accumulated over j-tiles — the
     ones column yields the softmax denominator row for free.
  4. Batched reciprocal of the 12 denominator rows, broadcast via a DRAM
     bounce, one normalize multiply per head pair.
  5. proj: out[n, c_out] = attnT.T @ pwT (natural layout, no final
     transpose), bias add on DVE, DMA out.
"""

import sys

if "/opt/trn_rl_repo" not in sys.path:
    sys.path.insert(0, "/opt/trn_rl_repo")

from contextlib import ExitStack

import numpy as np

import concourse.bass as bass
import concourse.tile as tile
from concourse import mybir
from concourse.bass_utils import run_bass_kernel_spmd

F32 = mybir.dt.float32
BF16 = mybir.dt.bfloat16
AF = mybir.ActivationFunctionType


def _split_dma_waits(nc: bass.Bass):
    """TRN2 instruction encodings hold at most 1 sync-wait (EventSemaphore: 2),
    but Tile can attach several (producer + xbar-mode serialization guards).
    Hoist all but one wait onto single-wait NoOps inserted just before on the
    same engine — same-sequencer FIFO order makes this equivalent.
    """
    for f in nc.m.functions:
        for blk in f.blocks:
            insts = blk.instructions
            i = 0
            while i < len(insts):
                inst = insts[i]
                limit = 2 if isinstance(inst, mybir.InstEventSemaphore) else 1
                if (inst.sync_info is not None
                        and len(inst.sync_info.on_wait) > limit):
                    waits = list(inst.sync_info.on_wait)
                    inst.sync_info = mybir.SyncInfo(
                        on_wait=waits[-limit:],
                        on_update=list(inst.sync_info.on_update))
                    for w in waits[:-limit]:
                        nop = mybir.InstNoOp(
                            name=nc.get_next_instruction_name(),
                            ins=[], outs=[])
                        nop.engine = inst.engine
                        nop.sync_info = mybir.SyncInfo(
                            on_wait=[w], on_update=[])
                        insts.insert(i, nop)
                        i += 1
                i += 1

B, N, C = 8, 1024, 768
H, HD = 12, 64
D3 = 3 * C  # 2304
SCALE = HD ** -0.5
NT = N // 128   # 8  token tiles
CT = C // 128   # 6  channel tiles
QT = D3 // 128  # 18 qkv row tiles


def build_kernel(nc: bass.Bass):
    x = nc.dram_tensor("x", [N, C], F32, kind="ExternalInput").ap()
    qkv_w = nc.dram_tensor("qkv_w", [D3, C], F32, kind="ExternalInput").ap()
    qkv_b = nc.dram_tensor("qkv_b", [D3], F32, kind="ExternalInput").ap()
    proj_w = nc.dram_tensor("proj_w", [C, C], F32, kind="ExternalInput").ap()
    proj_b = nc.dram_tensor("proj_b", [C], F32, kind="ExternalInput").ap()
    out = nc.dram_tensor("out", [N, C], F32, kind="ExternalOutput").ap()

    def bcast_ap(src: bass.AP, parts: int) -> bass.AP:
        # partition-broadcast a 1-D DRAM row: ap [[0, parts], [1, n]]
        return bass.AP(tensor=src.tensor, offset=src.offset,
                       ap=[[0, parts], *src.ap])

    with tile.TileContext(nc) as tc, ExitStack() as ctx:
        consts = ctx.enter_context(tc.tile_pool(name="consts", bufs=1))
        stage = ctx.enter_context(tc.tile_pool(name="stage", bufs=4))
        expp = ctx.enter_context(tc.tile_pool(name="expp", bufs=4))
        outp = ctx.enter_context(tc.tile_pool(name="outp", bufs=3))
        ps_s = ctx.enter_context(tc.tile_pool(name="ps_s", bufs=2, space="PSUM"))
        ps_o = ctx.enter_context(tc.tile_pool(name="ps_o", bufs=2, space="PSUM"))
        dram = ctx.enter_context(tc.tile_pool(name="dram", bufs=1, space="DRAM"))

        # ---- persistent bf16 operands --------------------------------
        xT = consts.tile([128, CT, N], BF16)        # x.T   [c, n]
        wT = consts.tile([128, CT, D3], BF16)       # qkv_w.T [c, d']
        pwT = consts.tile([128, CT, C], BF16)       # proj_w.T [c, c_out]
        qTt = consts.tile([128, CT, N], BF16)       # q.T  [d, n] (+bias)
        kTt = consts.tile([128, CT, N], BF16)       # k.T  [d, n] (+bias)
        v_sb = consts.tile([128, NT, H, HD + 1], BF16)  # v natural + ones col
        attnU = consts.tile([128, CT, N], F32)      # unnormalized attn.T
        attnT = consts.tile([128, CT, N], BF16)     # normalized attn.T
        qkb = consts.tile([128, 2 * CT], F32)       # q,k bias per-partition
        vb_bc = consts.tile([128, C], F32)          # v bias bcast
        pjb_bc = consts.tile([128, C], F32)         # proj bias bcast
        recip_all = consts.tile([H, N], F32)        # 1/denominator per head
        dscratch = dram.tile([H, N], F32)           # DRAM bounce for bcast

        # ---- biases ---------------------------------------------------
        # q,k bias laid out [p, t]: d' = t*128 + p  (t in 0..11)
        nc.sync.dma_start(out=qkb, in_=qkv_b[0:2 * C].rearrange(
            "(t p) -> p t", p=128))
        nc.sync.dma_start(out=vb_bc, in_=bcast_ap(qkv_b[2 * C:D3], 128))
        nc.sync.dma_start(out=pjb_bc, in_=bcast_ap(proj_b, 128))
        nc.vector.memset(v_sb[:, :, :, HD:HD + 1], 1.0)

        # ---- load + cast + transpose x and weights -------------------
        # casts on GpSimd (otherwise idle), transposes round-robin over
        # both HWDGE queues (sync + scalar) to overlap.
        prep = [(x, xT, t) for t in range(NT)]
        prep += [(qkv_w, wT, t) for t in range(QT)]
        prep += [(proj_w, pwT, t) for t in range(CT)]
        for i, (src, dstT, t) in enumerate(prep):
            xs = stage.tile([128, C], F32, tag="xs", name="xs")
            nc.sync.dma_start(out=xs, in_=src[t * 128:(t + 1) * 128, :])
            xb = stage.tile([128, C], BF16, tag="xb", name="xb")
            nc.gpsimd.tensor_copy(out=xb, in_=xs)
            eng = nc.sync if i % 2 == 0 else nc.scalar
            eng.dma_start_transpose(
                out=dstT[:, :, t * 128:(t + 1) * 128], in_=xb)

        # ---- QKV projection ------------------------------------------
        # Q and K transposed: qkvT[d', n] = wT.T @ xT, d' tiles 0..11
        for t in range(2 * CT):
            dst = qTt if t < CT else kTt
            tt = t % CT
            ps = ps_s.tile([128, N], F32, tag="ps")
            for ic in range(2):
                for ct in range(CT):
                    nc.tensor.matmul(
                        ps[:, ic * 512:(ic + 1) * 512],
                        lhsT=wT[:, ct, t * 128:(t + 1) * 128],
                        rhs=xT[:, ct, ic * 512:(ic + 1) * 512],
                        start=(ct == 0), stop=(ct == CT - 1))
            nc.vector.tensor_scalar_add(
                out=dst[:, tt, :], in0=ps, scalar1=qkb[:, t:t + 1])
        # V natural: v[n, dv] = xT.T @ wT[:, :, 1536:2304]
        for t in range(NT):
            psv = ps_s.tile([128, N], F32, tag="ps", name="psv")
            for lo, hi in ((0, 512), (512, 768)):
                for ct in range(CT):
                    nc.tensor.matmul(
                        psv[:, lo:hi],
                        lhsT=xT[:, ct, t * 128:(t + 1) * 128],
                        rhs=wT[:, ct, 2 * C + lo:2 * C + hi],
                        start=(ct == 0), stop=(ct == CT - 1))
            nc.vector.tensor_add(
                out=v_sb[:, t, :, 0:HD],
                in0=psv[:, 0:C].rearrange("p (h d) -> p h d", h=H),
                in1=vb_bc.rearrange("p (h d) -> p h d", h=H))

        # ---- attention (head pairs share a 128-partition tile) -------
        for h in range(H):
            t, base = h // 2, (h % 2) * 64
            o_ps = ps_o.tile([HD + 1, N], F32, tag="ops")

            def scores(jt):
                # software-pipelined: scores(jt+1) is emitted before pv(jt)
                # so exp waits never stall later scores in the PE FIFO
                s_ps = ps_s.tile([128, N], F32, tag="ps", name="s_ps")
                e = expp.tile([128, N], BF16, tag="e", name="e")
                for ic in range(2):
                    nc.tensor.matmul(
                        s_ps[:, ic * 512:(ic + 1) * 512],
                        lhsT=kTt[base:base + 64, t, jt * 128:(jt + 1) * 128],
                        rhs=qTt[base:base + 64, t, ic * 512:(ic + 1) * 512],
                        start=True, stop=True)
                nc.scalar.activation(out=e, in_=s_ps, func=AF.Exp,
                                     scale=SCALE)
                return e

            def pv(jt, e):
                for ic in range(2):
                    nc.tensor.matmul(
                        o_ps[:, ic * 512:(ic + 1) * 512],
                        lhsT=v_sb[:, jt, h, :],
                        rhs=e[:, ic * 512:(ic + 1) * 512],
                        start=(jt == 0), stop=(jt == NT - 1))

            e_prev = scores(0)
            for jt in range(1, NT):
                e_cur = scores(jt)
                pv(jt - 1, e_prev)
                e_prev = e_cur
            pv(NT - 1, e_prev)
            # unnormalized head output -> attnU rows [base, base+64)
            nc.vector.tensor_copy(
                out=attnU[base:base + 64, t, :], in_=o_ps[0:HD, :])
            # denominator row -> partition-64 staging -> recip_all[h, :]
            den = stage.tile([65, N], F32, tag="den")
            nc.vector.tensor_copy(
                out=den[HD:HD + 1, :], in_=o_ps[HD:HD + 1, :])
            nc.sync.dma_start(out=recip_all[h:h + 1, :],
                              in_=den[HD:HD + 1, :])

        # ---- normalize -----------------------------------------------
        nc.vector.reciprocal(out=recip_all, in_=recip_all)
        nc.sync.dma_start(out=dscratch, in_=recip_all)
        for t in range(CT):
            rbc = stage.tile([128, N], F32, tag="rbc")
            nc.sync.dma_start(out=rbc[0:64, :],
                              in_=bcast_ap(dscratch[2 * t, :], 64))
            nc.sync.dma_start(out=rbc[64:128, :],
                              in_=bcast_ap(dscratch[2 * t + 1, :], 64))
            nc.vector.tensor_mul(
                out=attnT[:, t, :], in0=attnU[:, t, :], in1=rbc)

        # ---- output projection ---------------------------------------
        for t in range(NT):
            osb = outp.tile([128, C], F32, tag="osb")
            pso = ps_s.tile([128, N], F32, tag="ps", name="pso")
            for lo, hi in ((0, 512), (512, 768)):
                for ct in range(CT):
                    nc.tensor.matmul(
                        pso[:, lo:hi],
                        lhsT=attnT[:, ct, t * 128:(t + 1) * 128],
                        rhs=pwT[:, ct, lo:hi],
                        start=(ct == 0), stop=(ct == CT - 1))
            nc.vector.tensor_add(out=osb, in0=pso[:, 0:C], in1=pjb_bc)
            nc.sync.dma_start(out=out[t * 128:(t + 1) * 128, :], in_=osb)

    _split_dma_waits(nc)
    return nc


_NC_CACHE = None


def _get_nc():
    global _NC_CACHE
    if _NC_CACHE is None:
        _NC_CACHE = build_kernel(
            bass.Bass("TRN2", target_bir_lowering=False, debug=False))
    return _NC_CACHE


def kernel(**inputs: np.ndarray) -> np.ndarray:
    nc = _get_nc()
    x = np.ascontiguousarray(inputs["x"], dtype=np.float32)
    shared = {
        "qkv_w": np.ascontiguousarray(inputs["qkv_w"], dtype=np.float32),
        "qkv_b": np.ascontiguousarray(inputs["qkv_b"], dtype=np.float32),
        "proj_w": np.ascontiguousarray(inputs["proj_w"], dtype=np.float32),
        "proj_b": np.ascontiguousarray(inputs["proj_b"], dtype=np.float32),
    }
    in_maps = [{"x": x[b], **shared} for b in range(B)]
    res = run_bass_kernel_spmd(nc, in_maps, core_ids=list(range(B)))
    return np.stack([r["out"] for r in res.results]).astype(np.float32)


if __name__ == "__main__":
    from reference import setup_inputs, reference

    inputs = {k: np.asarray(v) for k, v in setup_inputs().items()}
    got = kernel(**inputs)
    exp = np.asarray(reference(**inputs))
    err = np.abs(got - exp)
    print("abs err max:", err.max(), "ref absmax:", np.abs(exp).max())
    print("rel(absmax):", err.max() / np.abs(exp).max())
